# revision 10
# baseline (speedup 1.0000x reference)
"""GAT (2-layer, PyG-style) on 8 Trainium2 NeuronCores — v2.

Design (degree-binned slot layout, pair-packed gather rows, v2 scheduling):
- Node relabeling as v1: rank by in-degree, round-robin to 8 cores, each
  core's 6250 nodes degree-sorted so 128-node blocks are degree-homogeneous.
  Slot (core c, block b, partition p) owns one SBUF partition row; its
  in-edges lie along the free dim.
- Per-edge dma_gather of the 512B pair row [h_even|h_odd|a_s_even|a_s_odd];
  pair index 1+(src>>1) fits int16.
- v2 changes vs v1:
  * per-BLOCK gathers round-robined over all 4 SWDGE queues -> ~4x DMA
    concurrency (v1 serialized ~1.2 gathers at a time).
  * compute batched over uniform-width CHUNKS of blocks (nb x W_c slots);
    alpha path: one TT add + clamp + Lrelu(scalar) + Exp(scalar) + mask.
  * segment sums via contiguous in-place TREE adds (v1's strided
    tensor_reduce ran 6x off roofline).
  * chunk-wide epilogues (ELU / log-softmax) and one DMA per chunk for
    my-row / output writes.
  * stale gather-buffer columns (block width < chunk width) are killed by
    alpha clamp (+30) + multiplicative pad mask; pad slots within a block
    hit dummy row 0 whose a_s = -1e30.
- Layer1 -> layer2 node tables exchanged with AllGather; ONE NEFF, SPMD x8.
"""
import numpy as np
import ml_dtypes

# ---- problem constants (hardcoded per contest rules) ----
N = 50000
IN = 256
H1, F1 = 8, 8
HID = 64
OUT = 64
SLOPE = 0.2
NCORES = 8
P = 128
SHARD = N // NCORES            # 6250
NBLK = (SHARD + P - 1) // P    # 49
SHARD_PAD = NBLK * P           # 6272
NPAIR = N // 2                 # 25000
TROWS = NPAIR + 1              # +1 dummy pair row at 0
ROW = 256                      # bf16 elems per pair row (512B)
CC = 64                        # max cols per compute chunk
GMAX = 8192                    # max idx per gather instruction
NEG = -1e30
CLAMP = 30.0

_cache = {}


# ======================================================================
# host-side plan (pure index bookkeeping on edge_index)
# ======================================================================
def _build_plan(edge_index):
    src0 = edge_index[0].astype(np.int64)
    dst0 = edge_index[1].astype(np.int64)
    loops = np.arange(N, dtype=np.int64)
    src = np.concatenate([src0, loops])
    dst = np.concatenate([dst0, loops])

    indeg = np.bincount(dst, minlength=N)
    rank = np.argsort(indeg, kind="stable")          # rank r -> orig node
    r_of = np.empty(N, dtype=np.int64)
    r_of[rank] = np.arange(N)
    core_of = r_of % NCORES
    pos_of = r_of // NCORES
    new_of_orig = core_of * SHARD + pos_of
    orig_of_new = np.empty(N, dtype=np.int64)
    orig_of_new[new_of_orig] = np.arange(N)

    s = new_of_orig[src]
    d = new_of_orig[dst]

    # per-core CSR over local dst + global block widths
    cores = []
    W = np.zeros(NBLK, dtype=np.int64)
    for c in range(NCORES):
        m = (d // SHARD) == c
        sc = s[m]
        dc = d[m] - c * SHARD
        order = np.argsort(dc, kind="stable")
        sc, dc = sc[order], dc[order]
        deg = np.bincount(dc, minlength=SHARD)
        off = np.zeros(SHARD + 1, dtype=np.int64)
        np.cumsum(deg, out=off[1:])
        cores.append((sc, off, deg))
        degp = np.concatenate([deg, np.zeros(SHARD_PAD - SHARD, dtype=np.int64)])
        W = np.maximum(W, degp.reshape(NBLK, P).max(axis=1))
    W = np.maximum(W, 1)
    assert W.max() <= CC, f"block width {W.max()} exceeds CC={CC}"

    # gather stream offsets (real cols only, 2% pad)
    cum = np.zeros(NBLK + 1, dtype=np.int64)
    np.cumsum(W, out=cum[1:])
    SW = int(cum[-1])
    NIDX = SW * P

    # uniform-width compute chunks
    chunks = []                       # (b0, b1, W_c, ccol_off)
    ccum = 0
    b0 = 0
    while b0 < NBLK:
        b1 = b0 + 1
        while b1 < NBLK and (b1 + 1 - b0) * int(W[b0:b1 + 1].max()) <= CC:
            b1 += 1
        wc = int(W[b0:b1].max())
        chunks.append((b0, b1, wc, ccum))
        ccum += (b1 - b0) * wc
        b0 = b1
    SWC = ccum                        # total chunk-layout cols

    idx_streams, mask_streams = [], []
    for c in range(NCORES):
        sc, off, deg = cores[c]
        idx_blk = np.zeros((SW, P), dtype=np.int16)            # gather stream
        msk = np.zeros((SWC, P, 2), dtype=ml_dtypes.bfloat16)  # chunk layout
        for (b0, b1, wc, coff) in chunks:
            for b in range(b0, b1):
                gbase = int(cum[b])
                mbase = coff + (b - b0) * wc
                for p in range(P):
                    n = b * P + p
                    if n >= SHARD:
                        continue
                    es = sc[off[n]: off[n + 1]]
                    k = len(es)
                    idx_blk[gbase: gbase + k, p] = (es >> 1) + 1
                    par = (es & 1).astype(np.int64)
                    msk[mbase + np.arange(k), p, par] = 1.0
        # wrap idx stream: i = w*128+p -> [16, NIDX/16] col-major, replicate x8
        lin_idx = idx_blk.reshape(-1)
        wrapped = np.zeros((16, NIDX // 16), dtype=np.int16)
        ii = np.arange(NIDX)
        wrapped[ii % 16, ii // 16] = lin_idx
        idx_streams.append(np.tile(wrapped, (8, 1)))
        mask_streams.append(np.ascontiguousarray(msk.transpose(1, 0, 2)))  # [P, SWC, 2]

    return {
        "new_of_orig": new_of_orig,
        "orig_of_new": orig_of_new,
        "W": W, "cum": cum, "SW": SW, "NIDX": NIDX, "SWC": SWC,
        "chunks": chunks,
        "idx_streams": idx_streams,
        "mask_streams": mask_streams,
    }


# ======================================================================
# bass kernel build
# ======================================================================
def _build_nc(plan):
    import concourse.bacc as bacc
    import concourse.mybir as mybir
    import concourse.tile as tile
    from concourse.library_config import mlp
    from concourse.masks import make_identity

    f32, bf16, i16 = mybir.dt.float32, mybir.dt.bfloat16, mybir.dt.int16
    AF = mybir.ActivationFunctionType
    OP = mybir.AluOpType
    AX = mybir.AxisListType

    W = plan["W"]; cum = plan["cum"]; SW = plan["SW"]; NIDX = plan["NIDX"]
    SWC = plan["SWC"]; chunks = plan["chunks"]
    NBMAX = max(b1 - b0 for (b0, b1, _, _) in chunks)

    nc = bacc.Bacc("TRN2", debug=False, num_swdge_queues=4)

    xT = nc.dram_tensor("xT", [IN, SHARD_PAD], bf16, kind="ExternalInput")
    idxs = nc.dram_tensor("idxs", [P, NIDX // 16], i16, kind="ExternalInput")
    pmask = nc.dram_tensor("pmask", [P, SWC * 2], bf16, kind="ExternalInput")
    w1 = nc.dram_tensor("w1", [IN, HID], f32, kind="ExternalInput")
    as1 = nc.dram_tensor("as1", [1, HID], f32, kind="ExternalInput")
    ad1 = nc.dram_tensor("ad1", [1, HID], f32, kind="ExternalInput")
    b1v = nc.dram_tensor("b1v", [1, HID], f32, kind="ExternalInput")
    w2 = nc.dram_tensor("w2", [HID, OUT], f32, kind="ExternalInput")
    as2 = nc.dram_tensor("as2", [1, OUT], f32, kind="ExternalInput")
    ad2 = nc.dram_tensor("ad2", [1, OUT], f32, kind="ExternalInput")
    b2v = nc.dram_tensor("b2v", [1, OUT], f32, kind="ExternalInput")
    drow = nc.dram_tensor("drow", [1, ROW], bf16, kind="ExternalInput")
    out = nc.dram_tensor("out", [SHARD_PAD, OUT], f32, kind="ExternalOutput")

    table1 = nc.dram_tensor("table1", [TROWS, ROW], bf16)
    table2 = nc.dram_tensor("table2", [TROWS, ROW], bf16)
    my1 = nc.dram_tensor("my1", [SHARD_PAD // 2, ROW], bf16)
    my2 = nc.dram_tensor("my2", [SHARD_PAD // 2, ROW], bf16)

    core_ids = list(range(NCORES))
    qctr = [0]

    def next_q():
        q = qctr[0] % 4
        qctr[0] += 1
        return q

    with tile.TileContext(nc) as tc:
        with (
            tc.tile_pool(name="persist", bufs=1) as pp,
            tc.tile_pool(name="gbuf", bufs=3) as gp,
            tc.tile_pool(name="work", bufs=2) as wp,
            tc.tile_pool(name="psum", bufs=2, space="PSUM") as psp,
            tc.tile_pool(name="stage", bufs=2) as sp,
        ):
            nc.gpsimd.load_library(mlp)

            # ---------- persistent tiles ----------
            idx_t = pp.tile([P, NIDX // 16], i16)
            nc.sync.dma_start(idx_t[:], idxs[:])
            pm_t = pp.tile([P, SWC * 2], bf16)
            nc.sync.dma_start(pm_t[:], pmask[:])
            ident = pp.tile([P, P], f32)
            make_identity(nc, ident[:])
            ad1_all = pp.tile([P, NBLK * H1], f32)
            ad2_all = pp.tile([P, NBLK], f32)
            b1_bc = pp.tile([P, HID], f32)
            b2_bc = pp.tile([P, OUT], f32)

            small = pp.tile([1, HID], f32, tag="sm1")
            nc.sync.dma_start(small[:], b1v[:])
            nc.gpsimd.partition_broadcast(b1_bc[:], small[0:1, :])
            small2 = pp.tile([1, OUT], f32, tag="sm2")
            nc.sync.dma_start(small2[:], b2v[:])
            nc.gpsimd.partition_broadcast(b2_bc[:], small2[0:1, :])

            # ---------- W1aug = [W1 | A_s1 | A_d1] in bf16, 2 K-chunks ----------
            as1_bc = pp.tile([P, HID], f32, tag="as1b")
            sm = pp.tile([1, HID], f32, tag="sm3")
            nc.sync.dma_start(sm[:], as1[:])
            nc.gpsimd.partition_broadcast(as1_bc[:], sm[0:1, :])
            ad1_bc = pp.tile([P, HID], f32, tag="ad1b")
            sm2 = pp.tile([1, HID], f32, tag="sm4")
            nc.sync.dma_start(sm2[:], ad1[:])
            nc.gpsimd.partition_broadcast(ad1_bc[:], sm2[0:1, :])

            w1aug = []
            for k in range(2):
                w1c = wp.tile([P, HID], f32, tag="w1c")
                nc.sync.dma_start(w1c[:], w1[k * P:(k + 1) * P, :])
                aug = pp.tile([P, 80], bf16, tag=f"w1aug{k}")
                nc.vector.tensor_copy(out=aug[:, 0:HID], in_=w1c[:])
                tmp = wp.tile([P, HID], f32, tag="w1tmp")
                nc.vector.tensor_tensor(out=tmp[:], in0=w1c[:], in1=as1_bc[:], op=OP.mult)
                asr = wp.tile([P, H1], f32, tag="w1red")
                nc.vector.tensor_reduce(out=asr[:], in_=tmp[:].rearrange("p (h f) -> p h f", h=H1),
                                        op=OP.add, axis=AX.X)
                nc.vector.tensor_copy(out=aug[:, 64:72], in_=asr[:])
                nc.vector.tensor_tensor(out=tmp[:], in0=w1c[:], in1=ad1_bc[:], op=OP.mult)
                nc.vector.tensor_reduce(out=asr[:], in_=tmp[:].rearrange("p (h f) -> p h f", h=H1),
                                        op=OP.add, axis=AX.X)
                nc.vector.tensor_copy(out=aug[:, 72:80], in_=asr[:])
                w1aug.append(aug)

            # ---------- W2aug = [W2 | A_s2 | A_d2] [64, 66] bf16 ----------
            as2_bc = pp.tile([P, OUT], f32, tag="as2b")
            smb = pp.tile([1, OUT], f32, tag="sm5")
            nc.sync.dma_start(smb[:], as2[:])
            nc.gpsimd.partition_broadcast(as2_bc[:], smb[0:1, :])
            ad2_bc = pp.tile([P, OUT], f32, tag="ad2b")
            smc = pp.tile([1, OUT], f32, tag="sm6")
            nc.sync.dma_start(smc[:], ad2[:])
            nc.gpsimd.partition_broadcast(ad2_bc[:], smc[0:1, :])

            w2c = pp.tile([HID, OUT], f32, tag="w2c")
            nc.sync.dma_start(w2c[:], w2[:])
            w2aug = pp.tile([HID, 66], bf16, tag="w2aug")
            nc.vector.tensor_copy(out=w2aug[:, 0:OUT], in_=w2c[:])
            tmp2 = wp.tile([HID, OUT], f32, tag="w2tmp")
            nc.vector.tensor_tensor(out=tmp2[:], in0=w2c[:], in1=as2_bc[0:HID, :], op=OP.mult)
            red2 = wp.tile([HID, 1], f32, tag="w2red")
            nc.vector.tensor_reduce(out=red2[:], in_=tmp2[:], op=OP.add, axis=AX.X)
            nc.vector.tensor_copy(out=w2aug[:, 64:65], in_=red2[:])
            nc.vector.tensor_tensor(out=tmp2[:], in0=w2c[:], in1=ad2_bc[0:HID, :], op=OP.mult)
            nc.vector.tensor_reduce(out=red2[:], in_=tmp2[:], op=OP.add, axis=AX.X)
            nc.vector.tensor_copy(out=w2aug[:, 65:66], in_=red2[:])

            # ---------- stage 1: per chunk matmul -> my1 rows + a_d1 ----------
            nc.sync.dma_start(table1[0:1, :], drow[:])
            nc.sync.dma_start(table2[0:1, :], drow[:])
            for (b0, b1, wc, coff) in chunks:
                nb = b1 - b0
                xt = sp.tile([P, 2, NBMAX * P], bf16, tag="xt")
                for k in range(2):
                    nc.sync.dma_start(xt[:, k, 0:nb * P],
                                      xT[k * P:(k + 1) * P, b0 * P:b1 * P])
                for j in range(nb):
                    b = b0 + j
                    ps = psp.tile([P, 80], f32, tag="s1ps")
                    for k in range(2):
                        nc.tensor.matmul(ps[:], lhsT=xt[:, k, j * P:(j + 1) * P],
                                         rhs=w1aug[k][:], start=(k == 0), stop=(k == 1))
                    nc.vector.tensor_copy(out=ad1_all[:, b * H1:(b + 1) * H1],
                                          in_=ps[:, 72:80])
                    pkh = sp.tile([P, HID], bf16, tag="pkh")
                    nc.scalar.activation(pkh[:], ps[:, 0:HID], AF.Copy)
                    pka = sp.tile([P, H1], f32, tag="pka")
                    nc.vector.tensor_copy(out=pka[:], in_=ps[:, 64:72])
                    nc.sync.dma_start(
                        my1[b * 64:(b + 1) * 64, 0:128].rearrange(
                            "r (t f) -> r t f", t=2), pkh[:])
                    nc.sync.dma_start(
                        my1[b * 64:(b + 1) * 64, 128:160].bitcast(f32).rearrange(
                            "r (t a) -> r t a", t=2), pka[:])

            # ---------- AllGather layer-1 table ----------
            nc.gpsimd.collective_compute(
                "AllGather", mybir.AluOpType.bypass,
                replica_groups=[core_ids],
                ins=[my1[0:SHARD // 2, :]],
                outs=[table1[1:TROWS, :]],
            )

            # ---------- edge phase ----------
            def edge_phase(layer, table, ad_all):
                H = H1 if layer == 1 else 1
                for ci, (b0, b1, wc, coff) in enumerate(chunks):
                    nb = b1 - b0
                    ncol = nb * wc
                    nwt = wc * 2
                    g = gp.tile([P, CC, ROW], bf16, tag="g")
                    if layer == 1 and ci < 3:
                        # first use of each ring buffer: clear setup-phase
                        # garbage so stale cols can't be Inf/NaN pre-mask
                        nc.vector.memset(g[:], 0.0)
                    for j in range(nb):
                        b = b0 + j
                        wreal = int(W[b])
                        o = 0
                        while o < wreal:
                            take = min(wreal - o, GMAX // P)
                            nidx = take * P
                            nc.gpsimd.dma_gather(
                                g[:, j * wc + o: j * wc + o + take, :], table[:],
                                idx_t[:, (int(cum[b]) + o) * 8:(int(cum[b]) + o + take) * 8],
                                nidx, nidx, ROW,
                                single_packet=False, queue_num=next_q(),
                            )
                            o += take
                    # ---- alpha = clamp(asg + adb), lrelu, exp, mask ----
                    CH = ncol * 2 * H
                    alpha = wp.tile([P, CC * 2 * H1], f32, tag="alpha")
                    araw = (g[:, 0:ncol, 128:160] if layer == 1
                            else g[:, 0:ncol, 128:132]).bitcast(f32).rearrange(
                        "p c (t h) -> p c t h", t=2)
                    av = alpha[:, 0:CH].rearrange(
                        "p (b w t h) -> p b w t h", b=nb, t=2, h=H)
                    for j in range(nb):
                        adb = ad_all[:, (b0 + j) * H:(b0 + j + 1) * H][
                            :, None, None, :].to_broadcast([P, wc, 2, H])
                        nc.vector.tensor_tensor(
                            out=av[:, j], in0=araw[:, j * wc:(j + 1) * wc],
                            in1=adb, op=OP.add)
                    af = alpha[:, 0:CH]
                    nc.vector.tensor_scalar_min(out=af, in0=af, scalar1=CLAMP)
                    nc.scalar.activation(af, af, AF.Lrelu, alpha=SLOPE)
                    exm = wp.tile([P, CC * 2 * H1], bf16, tag="exm")
                    exf = exm[:, 0:CH]
                    nc.scalar.activation(exf, af, AF.Exp)
                    pmv = pm_t[:, 2 * coff:2 * (coff + ncol)].rearrange(
                        "p (c t) -> p c t", t=2)[:, :, :, None].to_broadcast(
                        [P, ncol, 2, H])
                    nc.vector.tensor_tensor(
                        out=exf.rearrange("p (c t h) -> p c t h", t=2, h=H),
                        in0=exf.rearrange("p (c t h) -> p c t h", t=2, h=H),
                        in1=pmv, op=OP.mult)
                    # ---- prod = h * coef (per parity: 4D APs) ----
                    prod = wp.tile([P, CC * 2 * 64], bf16, tag="prod")
                    for t in range(2):
                        pv = prod[:, 0:ncol * 128].rearrange(
                            "p (c t h f) -> p c t h f", c=ncol, t=2, h=H)[:, :, t]
                        hpt = g[:, 0:ncol, t * 64:(t + 1) * 64].rearrange(
                            "p c (h f) -> p c h f", h=H)
                        ext = exf.rearrange("p (c t h) -> p c t h", t=2, h=H)[
                            :, :, t, :, None].to_broadcast([P, ncol, H, 64 // H])
                        nc.vector.tensor_tensor(out=pv, in0=hpt, in1=ext, op=OP.mult)
                    # ---- msum tree over (w t) per block, in place ----
                    tv = prod[:, 0:ncol * 128].rearrange(
                        "p (b wt f) -> p b wt f", b=nb, f=64)
                    n = nwt
                    while n > 1:
                        m = (n + 1) // 2
                        nc.vector.tensor_tensor(
                            out=tv[:, :, 0:n - m, :], in0=tv[:, :, 0:n - m, :],
                            in1=tv[:, :, m:n, :], op=OP.add)
                        n = m
                    msum = tv[:, :, 0, :]                       # [P, nb, 64]
                    # ---- den tree (f32) ----
                    dtr = wp.tile([P, CC * H1], f32, tag="dtr")
                    ev = exf.rearrange("p (b wt h) -> p b wt h", b=nb, h=H)
                    dv = dtr[:, 0:nb * wc * H].rearrange(
                        "p (b x h) -> p b x h", b=nb, h=H)
                    n = nwt
                    m = (n + 1) // 2
                    nc.vector.tensor_tensor(
                        out=dv[:, :, 0:n - m, :], in0=ev[:, :, 0:n - m, :],
                        in1=ev[:, :, m:n, :], op=OP.add)
                    if n - m < m:  # odd: copy the un-paired slot
                        nc.vector.tensor_copy(out=dv[:, :, n - m:m, :],
                                              in_=ev[:, :, n - m:m, :])
                    n = m
                    while n > 1:
                        m = (n + 1) // 2
                        nc.vector.tensor_tensor(
                            out=dv[:, :, 0:n - m, :], in0=dv[:, :, 0:n - m, :],
                            in1=dv[:, :, m:n, :], op=OP.add)
                        n = m
                    den = dv[:, :, 0, :]                        # [P, nb, H]
                    rec = wp.tile([P, NBMAX * H1], f32, tag="rec")
                    nc.vector.reciprocal(out=rec[:, 0:nb * H].rearrange(
                        "p (b h) -> p b h", b=nb), in_=den)
                    # ---- ob = msum * rec ----
                    ob = wp.tile([P, NBMAX * 64], f32, tag="ob")
                    obv = ob[:, 0:nb * 64].rearrange("p (b h f) -> p b h f", b=nb, h=H)
                    rb = rec[:, 0:nb * H].rearrange("p (b h) -> p b h", b=nb)[
                        :, :, :, None].to_broadcast([P, nb, H, 64 // H])
                    nc.vector.tensor_tensor(
                        out=obv, in0=msum.rearrange("p b (h f) -> p b h f", h=H),
                        in1=rb, op=OP.mult)
                    obf = ob[:, 0:nb * 64]
                    if layer == 1:
                        # + b1, ELU -> h2; stage-2 transform per block
                        b1b = b1_bc[:, None, :].to_broadcast([P, nb, HID])
                        nc.vector.tensor_tensor(
                            out=obf.rearrange("p (b f) -> p b f", b=nb),
                            in0=obf.rearrange("p (b f) -> p b f", b=nb),
                            in1=b1b, op=OP.add)
                        mn = wp.tile([P, NBMAX * 64], f32, tag="mn")
                        mnf = mn[:, 0:nb * 64]
                        nc.vector.tensor_scalar_min(out=mnf, in0=obf, scalar1=0.0)
                        nc.scalar.activation(mnf, mnf, AF.Exp)
                        nc.vector.tensor_scalar_add(out=mnf, in0=mnf, scalar1=-1.0)
                        nc.vector.tensor_tensor(out=obf, in0=obf, in1=mnf, op=OP.max)
                        for j in range(nb):
                            b = b0 + j
                            psT = psp.tile([64, P], f32, tag="psT")
                            nc.tensor.transpose(psT[:], obf.rearrange(
                                "p (b f) -> p b f", b=nb)[:, j, :], ident[:])
                            h2T = sp.tile([64, P], bf16, tag="h2T")
                            nc.scalar.activation(h2T[:], psT[:], AF.Copy)
                            ps2 = psp.tile([P, 66], f32, tag="ps2")
                            nc.tensor.matmul(ps2[:], lhsT=h2T[:], rhs=w2aug[:],
                                             start=True, stop=True)
                            nc.vector.tensor_copy(out=ad2_all[:, b:b + 1],
                                                  in_=ps2[:, 65:66])
                            pk2h = sp.tile([P, 64], bf16, tag="pk2h")
                            nc.scalar.activation(pk2h[:], ps2[:, 0:64], AF.Copy)
                            pk2a = sp.tile([P, 1], f32, tag="pk2a")
                            nc.vector.tensor_copy(out=pk2a[:], in_=ps2[:, 64:65])
                            nc.sync.dma_start(
                                my2[b * 64:(b + 1) * 64, 0:128].rearrange(
                                    "r (t f) -> r t f", t=2), pk2h[:])
                            nc.sync.dma_start(
                                my2[b * 64:(b + 1) * 64, 128:132].bitcast(f32).rearrange(
                                    "r (t a) -> r t a", t=2), pk2a[:])
                    else:
                        # + b2, log_softmax, write out
                        b2b = b2_bc[:, None, :].to_broadcast([P, nb, OUT])
                        obv3 = obf.rearrange("p (b f) -> p b f", b=nb)
                        nc.vector.tensor_tensor(out=obv3, in0=obv3, in1=b2b, op=OP.add)
                        rmax = wp.tile([P, NBMAX], f32, tag="rmax")
                        nc.vector.tensor_reduce(
                            out=rmax[:, 0:nb], in_=obv3, op=OP.max, axis=AX.X)
                        rmb = rmax[:, 0:nb][:, :, None].to_broadcast([P, nb, OUT])
                        esc = wp.tile([P, NBMAX * 64], bf16, tag="esc")
                        ev3 = esc[:, 0:nb * 64].rearrange("p (b f) -> p b f", b=nb)
                        nc.vector.tensor_tensor(out=ev3, in0=obv3, in1=rmb,
                                                op=OP.subtract)
                        nc.scalar.activation(esc[:, 0:nb * 64], esc[:, 0:nb * 64],
                                             AF.Exp)
                        rsum = wp.tile([P, NBMAX], f32, tag="rsum")
                        nc.vector.tensor_reduce(
                            out=rsum[:, 0:nb], in_=ev3, op=OP.add, axis=AX.X)
                        lns = wp.tile([P, NBMAX], f32, tag="lns")
                        nc.scalar.activation(lns[:, 0:nb], rsum[:, 0:nb], AF.Ln)
                        shift = wp.tile([P, NBMAX], f32, tag="shift")
                        nc.vector.tensor_tensor(out=shift[:, 0:nb], in0=rmax[:, 0:nb],
                                                in1=lns[:, 0:nb], op=OP.add)
                        fin = wp.tile([P, NBMAX * 64], f32, tag="fin")
                        fv3 = fin[:, 0:nb * 64].rearrange("p (b f) -> p b f", b=nb)
                        shb = shift[:, 0:nb][:, :, None].to_broadcast([P, nb, OUT])
                        nc.vector.tensor_tensor(out=fv3, in0=obv3, in1=shb,
                                                op=OP.subtract)
                        nc.sync.dma_start(
                            out[b0 * P:b1 * P, :].rearrange(
                                "(b p) f -> p b f", b=nb), fv3)

            edge_phase(1, table1, ad1_all)
            nc.gpsimd.collective_compute(
                "AllGather", mybir.AluOpType.bypass,
                replica_groups=[core_ids],
                ins=[my2[0:SHARD // 2, :]],
                outs=[table2[1:TROWS, :]],
            )
            edge_phase(2, table2, ad2_all)

    nc.finalize()
    return nc


# ======================================================================
# entry point
# ======================================================================
def kernel(**inputs):
    x = np.asarray(inputs["x"], dtype=np.float32)
    edge_index = np.asarray(inputs["edge_index"])
    W1 = np.asarray(inputs["W1"], dtype=np.float32)
    att_src1 = np.asarray(inputs["att_src1"], dtype=np.float32)
    att_dst1 = np.asarray(inputs["att_dst1"], dtype=np.float32)
    b1 = np.asarray(inputs["b1"], dtype=np.float32)
    W2 = np.asarray(inputs["W2"], dtype=np.float32)
    att_src2 = np.asarray(inputs["att_src2"], dtype=np.float32)
    att_dst2 = np.asarray(inputs["att_dst2"], dtype=np.float32)
    b2 = np.asarray(inputs["b2"], dtype=np.float32)

    key = hash(edge_index.tobytes())
    if key not in _cache:
        plan = _build_plan(edge_index)
        nc = _build_nc(plan)
        _cache[key] = (plan, nc)
    plan, nc = _cache[key]

    # ---- stage inputs ----
    orig_of_new = plan["orig_of_new"]
    new_of_orig = plan["new_of_orig"]
    x_new = x[orig_of_new]
    dummy = np.zeros(ROW, dtype=ml_dtypes.bfloat16)
    dummy.view(np.float32)[64:80] = NEG      # a_s slots (elems 128:160)
    dummy = dummy[None, :]

    in_maps = []
    for c in range(NCORES):
        xs = x_new[c * SHARD:(c + 1) * SHARD]
        xs = np.concatenate([xs, np.zeros((SHARD_PAD - SHARD, IN), np.float32)], axis=0)
        xTc = np.ascontiguousarray(xs.T).astype(ml_dtypes.bfloat16)
        in_maps.append({
            "xT": xTc.view(np.uint16),
            "idxs": plan["idx_streams"][c],
            "pmask": plan["mask_streams"][c].reshape(P, -1).view(np.uint16),
            "w1": W1, "as1": att_src1.reshape(1, -1), "ad1": att_dst1.reshape(1, -1),
            "b1v": b1.reshape(1, -1),
            "w2": W2, "as2": att_src2.reshape(1, -1), "ad2": att_dst2.reshape(1, -1),
            "b2v": b2.reshape(1, -1),
            "drow": dummy.view(np.uint16),
        })

    global _last_in_maps
    _last_in_maps = in_maps
    from concourse.bass_utils import run_bass_kernel_spmd
    res = run_bass_kernel_spmd(nc, in_maps, core_ids=list(range(NCORES)))

    full = np.zeros((N, OUT), dtype=np.float32)
    for c in range(NCORES):
        full[c * SHARD:(c + 1) * SHARD] = res.results[c]["out"][0:SHARD]
    return full[new_of_orig]


if __name__ == "__main__":
    d = np.load("/root/problem/ref_inputs.npz")
    outp = kernel(**{k: d[k] for k in d.files})
    exp = np.load("/root/problem/ref_out.npy")
    err = np.abs(outp - exp)
    print("max abs err:", err.max(), "rel:", err.max() / np.abs(exp).max())
